# revision 19
# baseline (speedup 1.0000x reference)
"""Causal attention with KV cache — Trainium2 Bass kernel, 8-core SPMD.

Sharding: batch (2) x head-group (4 heads each) = 8 cores.
Each core computes, for its (batch b, heads 4g..4g+3):
  Q/K/V projections (bf16 matmuls, fp32 psum),
  streaming softmax(QK^T)V with the KV cache (no max subtraction -- scores
  are O(5) here so exp is safe), and its partial output projection
  y_partial = attn_out @ wo[:, heads].T  (shape [T, C], fp32).
Host sums the 4 head-group partials per batch.

Device data layouts (host pre-arranges everything, bf16):
  x    [128 ci, CO co, T t]        = x[b].T  split c=(co ci)
  wq/wk/wv [128 ci, CO co, DL d]   = w[rows].T split c, (wq pre-scaled by hd^-0.5)
  wo   [128 di, HL do, C c]        = wo[:, rows].T split d=(do di)
  kc   [128 d, HL h, SCO so, 128 si]
  vc   [128 si, SCO so, HL h, 128 d]
  mk   [128 si, 4 m, 512 t]        causal masks for the 4 diagonal chunks
Output:
  y    [T, C] fp32 (partial sum over this core's heads)
"""

import os
import sys

import numpy as np

for _p in ("/opt/trn_rl_repo", os.path.expanduser("~/.axon_site/_ro/trn_rl_repo")):
    if _p not in sys.path and os.path.isdir(_p):
        sys.path.insert(0, _p)

import ml_dtypes  # noqa: E402

import concourse.bass as bass  # noqa: E402
import concourse.tile as tile  # noqa: E402
from concourse import mybir  # noqa: E402

BF16 = mybir.dt.bfloat16
F32 = mybir.dt.float32
P = 128

# Full-size problem constants
B, T, C, H, HD, START = 2, 2048, 2048, 16, 128, 1024
N_CORES = 8
N_GROUPS = N_CORES // B     # 4 head groups per batch
HL = H // N_GROUPS          # 4 local heads per core
TCH = 512  # t-chunk (psum free dim)


def build_nc(T_=T, C_=C, HL_=HL, SC_=START):
    """Build the per-core Bass module. All cores run the identical program."""
    nc = bass.Bass("TRN2", target_bir_lowering=False)

    CO = C_ // P            # contraction chunks for projections
    DL = HL_ * HD           # local head dims (512)
    NT = T_ // TCH          # query t-chunks
    TSUB = TCH // P         # 128-row subchunks per t-chunk (= #diagonal masks)
    SCO = SC_ // P          # cache s-chunks
    NCC = C_ // TCH         # output column chunks

    x_d = nc.dram_tensor("x", [P, CO, T_], BF16, kind="ExternalInput")
    wq_d = nc.dram_tensor("wq", [P, CO, DL], BF16, kind="ExternalInput")
    wk_d = nc.dram_tensor("wk", [P, CO, DL], BF16, kind="ExternalInput")
    wv_d = nc.dram_tensor("wv", [P, CO, DL], BF16, kind="ExternalInput")
    wo_d = nc.dram_tensor("wo", [P, HL_, C_], BF16, kind="ExternalInput")
    kc_d = nc.dram_tensor("kc", [P, HL_, SCO, P], BF16, kind="ExternalInput")
    vc_d = nc.dram_tensor("vc", [P, SCO, HL_, P], BF16, kind="ExternalInput")
    mk_d = nc.dram_tensor("mk", [P, TSUB, TCH], BF16, kind="ExternalInput")
    y_d = nc.dram_tensor("y", [T_, C_], F32, kind="ExternalOutput")

    F32R = mybir.dt.float32r
    with tile.TileContext(nc) as tc:
        with (
            tc.tile_pool(name="consts", bufs=1) as consts,
        ):
            # ---- persistent SBUF state (alive across both phases) ----
            ones_col = consts.tile([P, 1], BF16)
            nc.vector.memset(ones_col[:], 1.0)

            qt_sb = consts.tile([P, HL_, T_], BF16)   # Q^T  [d, h, t]
            kt_sb = consts.tile([P, HL_, T_], BF16)   # K^T new  [d, h, t]
            vn_sb = consts.tile([P, T_ // P, DL], BF16)  # V new  [t_i, t_o, d]
            kc_sb = consts.tile([P, HL_, SCO, P], BF16)
            vc_sb = consts.tile([P, SCO, HL_, P], BF16)
            mk_sb = consts.tile([P, TSUB, TCH], BF16)

            # ---- phase A: projections (x and wq/wk/wv freed afterwards) ----
            with tc.tile_pool(name="projp", bufs=1) as projp:
                xt_sb = projp.tile([P, CO, T_], BF16)
                wq_sb = projp.tile([P, CO, DL], BF16)
                wk_sb = projp.tile([P, CO, DL], BF16)
                wv_sb = projp.tile([P, CO, DL], BF16)
                # interleave wq/x chunk DMAs so the co-outer first Q sweep can
                # begin after chunk 0 lands rather than after the full 10MB
                for co in range(CO):
                    nc.sync.dma_start(out=wq_sb[:, co:co + 1, :],
                                      in_=wq_d[:, co:co + 1, :])
                    nc.sync.dma_start(out=xt_sb[:, co:co + 1, :],
                                      in_=x_d[:, co:co + 1, :])
                nc.sync.dma_start(out=wk_sb[:], in_=wk_d[:])
                nc.sync.dma_start(out=wv_sb[:], in_=wv_d[:])
                nc.sync.dma_start(out=kc_sb[:], in_=kc_d[:])
                nc.sync.dma_start(out=vc_sb[:], in_=vc_d[:])
                nc.sync.dma_start(out=mk_sb[:], in_=mk_d[:])

                # Q first half: co-outer over 8 concurrently-open psum groups,
                # paced by the x-chunk DMA arrivals (own scoped pool so the 8
                # banks are returned before the main psum pool allocates)
                with tc.tile_pool(name="psum8", bufs=1, space="PSUM") as psum8:
                    groups8 = [(h, ti) for h in range(HL_) for ti in range(2)]
                    pps8 = {}
                    for g in groups8:
                        pps8[g] = psum8.tile([P, TCH], F32, tag="mm8", bufs=8,
                                             name="pp8")
                    for co in range(CO):
                        for (h, ti) in groups8:
                            nc.tensor.matmul(
                                pps8[(h, ti)][:],
                                wq_sb[:, co, h * HD:(h + 1) * HD],
                                xt_sb[:, co, ti * TCH:(ti + 1) * TCH],
                                start=(co == 0),
                                stop=(co == CO - 1),
                            )
                    for (h, ti) in groups8:
                        nc.scalar.copy(out=qt_sb[:, h, ti * TCH:(ti + 1) * TCH],
                                       in_=pps8[(h, ti)][:])

                with tc.tile_pool(name="psumA", bufs=1, space="PSUM") as psumA:
                    # Q second half and K: co-inner per group (x resident)
                    qk_rest = [(wq_sb, qt_sb, h, ti)
                               for h in range(HL_) for ti in (2, 3)]
                    qk_rest += [(wk_sb, kt_sb, h, ti)
                                for h in range(HL_) for ti in range(NT)]
                    for wsb, dst, h, ti in qk_rest:
                        pp = psumA.tile([P, TCH], F32, tag="mm", bufs=2,
                                        name="pp_qk")
                        for co in range(CO):
                            nc.tensor.matmul(
                                pp[:],
                                wsb[:, co, h * HD:(h + 1) * HD],
                                xt_sb[:, co, ti * TCH:(ti + 1) * TCH],
                                start=(co == 0),
                                stop=(co == CO - 1),
                            )
                        nc.scalar.copy(
                            out=dst[:, h, ti * TCH:(ti + 1) * TCH], in_=pp[:]
                        )

                    # V in [t, d] layout: psum [t=128, d=DL]
                    for tt in range(T_ // P):
                        pv = psumA.tile([P, DL], F32, tag="mm", bufs=2,
                                        name="pv")
                        for co in range(CO):
                            nc.tensor.matmul(
                                pv[:],
                                xt_sb[:, co, tt * P:(tt + 1) * P],
                                wv_sb[:, co, :],
                                start=(co == 0),
                                stop=(co == CO - 1),
                            )
                        nc.scalar.copy(out=vn_sb[:, tt, :], in_=pv[:])

            # ---- phase B: attention + output projection ----
            with (
                tc.tile_pool(name="work", bufs=1) as work,
                tc.tile_pool(name="psum", bufs=1, space="PSUM") as psum,
                tc.tile_pool(name="dscr", bufs=2, space="DRAM") as dscr,
            ):
                wo_sb = work.tile([P, HL_, C_], BF16, name="wo_sb")
                nc.sync.dma_start(out=wo_sb[:], in_=wo_d[:])

                def emit_y(ti):
                    # y[t, c] += onorm^T(h-chunks) @ wo ; psum DMA'd straight out
                    onorm = onorms[ti]
                    for ci in range(NCC):
                        for tsub in range(TSUB):
                            py = psum.tile([P, TCH], F32, tag="mm", bufs=2,
                                           name="py")
                            for h in range(HL_):
                                nc.tensor.matmul(
                                    py[:],
                                    onorm[:, h, tsub * P:(tsub + 1) * P],
                                    wo_sb[:, h, ci * TCH:(ci + 1) * TCH],
                                    start=(h == 0),
                                    stop=(h == HL_ - 1),
                                )
                            ysb = work.tile([P, TCH], F32, tag="ysb", bufs=3,
                                            name="ysb")
                            nc.vector.tensor_copy(out=ysb[:], in_=py[:])
                            t0 = ti * TCH + tsub * P
                            nc.sync.dma_start(
                                out=y_d[t0:t0 + P, ci * TCH:(ci + 1) * TCH],
                                in_=ysb[:],
                            )

                onorms = {}
                for ti in range(NT):
                    onorm = work.tile([P, HL_, TCH], BF16, tag="onorm", bufs=2,
                                      name="onorm")
                    onorms[ti] = onorm
                    den4 = work.tile([P, TCH], F32, tag="den4", bufs=2,
                                     name="den4")
                    nc.vector.memset(den4[:], 1.0)
                    osbs = {}
                    for h in range(HL_):
                        n_new = (ti + 1) * TSUB
                        n_s = SCO + n_new
                        n_pair = n_s // 2
                        diag0 = n_s - TSUB  # first diagonal (masked) chunk
                        q_rhs = qt_sb[:, h, ti * TCH:(ti + 1) * TCH]

                        acc = psum.tile([P, TCH], F32, tag="acc", bufs=2,
                                        name="acc")
                        # running denominator partial-sums, split between DVE
                        # (chunks j%3 in {0,1}) and GpSimd (j%3 == 2)
                        paccd = work.tile([P, TCH], BF16, tag="paccd", bufs=2,
                                          name="paccd")
                        paccg = work.tile([P, TCH], BF16, tag="paccg", bufs=2,
                                          name="paccg")

                        def c0_of(j):
                            # first needed query column for key chunk j
                            # (causality: chunk at diag offset m only feeds
                            #  queries t >= 128*m)
                            return 0 if j < diag0 else P * (j - diag0)

                        def kt_of(j):
                            if j < SCO:
                                return kc_sb[:, h, j, :]
                            sn = j - SCO
                            return kt_sb[:, h, sn * P:(sn + 1) * P]

                        def v_of(j):
                            if j < SCO:
                                return vc_sb[:, j, h, :]
                            sn = j - SCO
                            return vn_sb[:, sn, h * HD:(h + 1) * HD]

                        def qk_pair(p):
                            s2 = psum.tile([P, 2, TCH], F32, tag="S2", bufs=2,
                                           name="s2")
                            for i in (0, 1):
                                j = 2 * p + i
                                c0 = c0_of(j)
                                nc.tensor.matmul(s2[:, i, c0:], kt_of(j),
                                                 q_rhs[:, c0:],
                                                 start=True, stop=True)
                            return s2

                        def consume_pair(p, s2):
                            e2 = work.tile([P, 2, TCH], BF16, tag="E", bufs=3,
                                           name="e2")
                            j0, j1 = 2 * p, 2 * p + 1
                            if c0_of(j1) == 0:
                                nc.scalar.activation(
                                    out=e2[:], in_=s2[:],
                                    func=mybir.ActivationFunctionType.Exp,
                                )
                            else:
                                for i, j in ((0, j0), (1, j1)):
                                    c0 = c0_of(j)
                                    nc.scalar.activation(
                                        out=e2[:, i, c0:], in_=s2[:, i, c0:],
                                        func=mybir.ActivationFunctionType.Exp,
                                    )
                            for i, j in ((0, j0), (1, j1)):
                                m = j - diag0
                                if m >= 0:
                                    # only the 128-wide diagonal block needs
                                    # masking; columns beyond it are all-ones
                                    c0 = P * m
                                    nc.vector.tensor_mul(
                                        e2[:, i, c0:c0 + P], e2[:, i, c0:c0 + P],
                                        mk_sb[:, m, c0:c0 + P])
                            for i, j in ((0, j0), (1, j1)):
                                c0 = c0_of(j)
                                if j % 3 == 2:
                                    if j == 2:
                                        nc.gpsimd.tensor_copy(
                                            out=paccg[:], in_=e2[:, i, :])
                                    else:
                                        nc.gpsimd.tensor_add(
                                            out=paccg[:, c0:],
                                            in0=paccg[:, c0:],
                                            in1=e2[:, i, c0:])
                                else:
                                    if j == 0:
                                        nc.vector.tensor_copy(out=paccd[:],
                                                              in_=e2[:, 0, :])
                                    else:
                                        nc.vector.tensor_add(paccd[:, c0:],
                                                             paccd[:, c0:],
                                                             e2[:, i, c0:])
                                nc.tensor.matmul(
                                    acc[:, c0:], v_of(j), e2[:, i, c0:],
                                    start=(j == 0), stop=(j == n_s - 1),
                                )

                        # software pipeline: QK(p+1) issued before consuming p
                        s2_prev = qk_pair(0)
                        for p in range(n_pair):
                            s2_next = qk_pair(p + 1) if p + 1 < n_pair else None
                            consume_pair(p, s2_prev)
                            s2_prev = s2_next

                        # release acc early: unnormalized output to sbuf
                        osb = work.tile([P, TCH], BF16, tag="osb", bufs=8,
                                        name="osb")
                        nc.vector.tensor_copy(out=osb[:], in_=acc[:])
                        osbs[h] = osb

                        # denominator: two ones-matmuls over the running sums
                        dps = psum.tile([1, TCH], F32, tag="mm", bufs=2,
                                        name="dps")
                        nc.tensor.matmul(dps[:1, :], ones_col[:], paccd[:],
                                         start=True, stop=False)
                        nc.tensor.matmul(dps[:1, :], ones_col[:], paccg[:],
                                         start=False, stop=True)
                        nc.scalar.copy(out=den4[32 * h:32 * h + 1, :], in_=dps[:1, :])

                    # normalization chain issued BEFORE emit_y(ti-1) so its
                    # DMA broadcast roundtrip hides under the py matmuls
                    recip4 = work.tile([P, TCH], F32, tag="recip4", bufs=2,
                                       name="recip4")
                    nc.vector.reciprocal(out=recip4[:], in_=den4[:])
                    # gather the 4 rows (partitions 0/32/64/96) to DRAM, then
                    # one broadcast read back (0-stride partition APs need DRAM)
                    rdr4 = dscr.tile([HL_, TCH], F32, tag="rdr4", bufs=2,
                                     name="rdr4")
                    nc.sync.dma_start(out=rdr4[:],
                                      in_=recip4[:32 * HL_:32, :])
                    rbc4 = work.tile([P, HL_, TCH], F32, tag="rbc4", bufs=2,
                                     name="rbc4")
                    bcast_src = bass.AP(
                        tensor=rdr4.tensor, offset=rdr4.offset,
                        ap=[[0, P], [TCH, HL_], [1, TCH]],
                    )
                    nc.sync.dma_start(out=rbc4[:], in_=bcast_src)
                    for h in range(HL_):
                        nc.vector.tensor_mul(onorm[:, h, :], osbs[h][:],
                                             rbc4[:, h, :])

                    # y of the previous t-chunk: fills PE while this chunk's
                    # normalization tail drains on DVE/DMA
                    if ti > 0:
                        emit_y(ti - 1)
                emit_y(NT - 1)

    # walrus allows a single sync wait per hw instruction: shed matmul extras
    # onto ldweights, then split any remaining multi-waits via event sems
    bass._bass_rust.move_matmul_waits_to_ldweights(nc.m)
    bass._bass_rust.generate_event_semaphores(nc)
    return nc


def _bf16(a):
    return np.ascontiguousarray(a).astype(ml_dtypes.bfloat16)


def make_core_inputs(x, k_cache, v_cache, wq, wk, wv, wo, core,
                     T_=T, C_=C, HL_=HL, SC_=START, n_groups=None):
    """Host-side shard + relayout for one core."""
    CO = C_ // P
    DL = HL_ * HD
    TSUB = TCH // P
    SCO = SC_ // P
    if n_groups is None:
        n_groups = (k_cache.shape[1] + HL_ - 1) // HL_
    b, g = divmod(core, n_groups)
    heads = slice(HL_ * g, HL_ * (g + 1))
    rows = slice(DL * g, DL * (g + 1))
    scale = HD ** -0.5

    xd = x[b].T.reshape(CO, P, T_).transpose(1, 0, 2)
    wqd = (wq[rows].T * scale).reshape(CO, P, DL).transpose(1, 0, 2)
    wkd = wk[rows].T.reshape(CO, P, DL).transpose(1, 0, 2)
    wvd = wv[rows].T.reshape(CO, P, DL).transpose(1, 0, 2)
    wod = wo[:, rows].T.reshape(HL_, P, C_).transpose(1, 0, 2)
    kcd = k_cache[b, heads].reshape(HL_, SCO, P, P).transpose(3, 0, 1, 2)
    vcd = v_cache[b, heads].reshape(HL_, SCO, P, P).transpose(2, 1, 0, 3)
    si = np.arange(P)[:, None, None]
    mm = np.arange(TSUB)[None, :, None]
    tt = np.arange(TCH)[None, None, :]
    mkd = (tt >= si + P * mm)

    return {
        "x": _bf16(xd), "wq": _bf16(wqd), "wk": _bf16(wkd), "wv": _bf16(wvd),
        "wo": _bf16(wod), "kc": _bf16(kcd), "vc": _bf16(vcd),
        "mk": _bf16(mkd.astype(np.float32)),
    }


_NC_CACHE = None


def _get_nc():
    global _NC_CACHE
    if _NC_CACHE is None:
        _NC_CACHE = build_nc()
    return _NC_CACHE


def run_spmd(inputs, trace=False):
    """Run the 8-core SPMD kernel; returns (y_full, BassKernelResults)."""
    from concourse.bass_utils import run_bass_kernel_spmd

    x = np.asarray(inputs["x"], dtype=np.float32)
    k_cache = np.asarray(inputs["k_cache"], dtype=np.float32)
    v_cache = np.asarray(inputs["v_cache"], dtype=np.float32)
    wq = np.asarray(inputs["wq"], dtype=np.float32)
    wk = np.asarray(inputs["wk"], dtype=np.float32)
    wv = np.asarray(inputs["wv"], dtype=np.float32)
    wo = np.asarray(inputs["wo"], dtype=np.float32)
    assert int(inputs["start_pos"]) == START

    nc = _get_nc()
    in_maps = [
        make_core_inputs(x, k_cache, v_cache, wq, wk, wv, wo, core)
        for core in range(N_CORES)
    ]
    res = run_bass_kernel_spmd(
        nc, in_maps, core_ids=list(range(N_CORES)), trace=trace
    )
    n_groups = N_CORES // B
    y = np.zeros((B, T, C), dtype=np.float32)
    for core in range(N_CORES):
        b = core // n_groups
        y[b] += np.asarray(res.results[core]["y"], dtype=np.float32)
    return y, res


def kernel(**inputs):
    y, _ = run_spmd(inputs, trace=False)
    return y



# revision 22
# speedup vs baseline: 1.0514x; 1.0514x over previous
"""Causal attention with KV cache — Trainium2 Bass kernel, 8-core SPMD.

Sharding: batch (2) x head-group (4 heads each) = 8 cores.
Each core computes, for its (batch b, heads 4g..4g+3):
  Q/K/V projections (bf16 matmuls, fp32 psum),
  streaming softmax(QK^T)V with the KV cache (no max subtraction -- scores
  are O(5) here so exp is safe), and its partial output projection
  y_partial = attn_out @ wo[:, heads].T  (shape [T, C], fp32).
Host sums the 4 head-group partials per batch.

Device data layouts (host pre-arranges everything, bf16):
  x    [128 ci, CO co, T t]        = x[b].T  split c=(co ci)
  wq/wk/wv [128 ci, CO co, DL d]   = w[rows].T split c, (wq pre-scaled by hd^-0.5)
  wo   [128 di, HL do, C c]        = wo[:, rows].T split d=(do di)
  kc   [128 d, HL h, SCO so, 128 si]
  vc   [128 si, SCO so, HL h, 128 d]
  mk   [128 si, 4 m, 512 t]        causal masks for the 4 diagonal chunks
Output:
  y    [T, C] fp32 (partial sum over this core's heads)
"""

import os
import sys

import numpy as np

for _p in ("/opt/trn_rl_repo", os.path.expanduser("~/.axon_site/_ro/trn_rl_repo")):
    if _p not in sys.path and os.path.isdir(_p):
        sys.path.insert(0, _p)

import ml_dtypes  # noqa: E402

import concourse.bass as bass  # noqa: E402
import concourse.tile as tile  # noqa: E402
from concourse import mybir  # noqa: E402

BF16 = mybir.dt.bfloat16
F32 = mybir.dt.float32
P = 128

# Full-size problem constants
B, T, C, H, HD, START = 2, 2048, 2048, 16, 128, 1024
N_CORES = 8
N_GROUPS = N_CORES // B     # 4 head groups per batch
HL = H // N_GROUPS          # 4 local heads per core
TCH = 512  # t-chunk (psum free dim)


def build_nc(T_=T, C_=C, HL_=HL, SC_=START):
    """Build the per-core Bass module. All cores run the identical program."""
    nc = bass.Bass("TRN2", target_bir_lowering=False)

    CO = C_ // P            # contraction chunks for projections
    DL = HL_ * HD           # local head dims (512)
    NT = T_ // TCH          # query t-chunks
    TSUB = TCH // P         # 128-row subchunks per t-chunk (= #diagonal masks)
    SCO = SC_ // P          # cache s-chunks
    NCC = C_ // TCH         # output column chunks

    x_d = nc.dram_tensor("x", [P, CO, T_], BF16, kind="ExternalInput")
    wq_d = nc.dram_tensor("wq", [P, CO, DL], BF16, kind="ExternalInput")
    wk_d = nc.dram_tensor("wk", [P, CO, DL], BF16, kind="ExternalInput")
    wv_d = nc.dram_tensor("wv", [P, CO, DL], BF16, kind="ExternalInput")
    wo_d = nc.dram_tensor("wo", [P, HL_, C_], BF16, kind="ExternalInput")
    kc_d = nc.dram_tensor("kc", [P, HL_, SCO, P], BF16, kind="ExternalInput")
    vc_d = nc.dram_tensor("vc", [P, SCO, HL_, P], BF16, kind="ExternalInput")
    mk_d = nc.dram_tensor("mk", [P, TSUB, TCH], BF16, kind="ExternalInput")
    y_d = nc.dram_tensor("y", [T_, C_], F32, kind="ExternalOutput")

    F32R = mybir.dt.float32r
    with tile.TileContext(nc) as tc:
        with (
            tc.tile_pool(name="consts", bufs=1) as consts,
        ):
            # ---- persistent SBUF state (alive across both phases) ----
            ones_col = consts.tile([P, 1], BF16)
            nc.vector.memset(ones_col[:], 1.0)

            qt_sb = consts.tile([P, HL_, T_], BF16)   # Q^T  [d, h, t]
            kt_sb = consts.tile([P, HL_, T_], BF16)   # K^T new  [d, h, t]
            vn_sb = consts.tile([P, T_ // P, DL], BF16)  # V new  [t_i, t_o, d]
            kc_sb = consts.tile([P, HL_, SCO, P], BF16)
            vc_sb = consts.tile([P, SCO, HL_, P], BF16)
            mk_sb = consts.tile([P, TSUB, TCH], BF16)

            # ---- phase A: projections (x and wq/wk/wv freed afterwards) ----
            with tc.tile_pool(name="projp", bufs=1) as projp:
                xt_sb = projp.tile([P, CO, T_], BF16)
                wq_sb = projp.tile([P, CO, DL], BF16)
                wk_sb = projp.tile([P, CO, DL], BF16)
                wv_sb = projp.tile([P, CO, DL], BF16)
                # interleave wq/x chunk DMAs so the co-outer first Q sweep can
                # begin after chunk 0 lands rather than after the full 10MB
                for co in range(CO):
                    nc.sync.dma_start(out=wq_sb[:, co:co + 1, :],
                                      in_=wq_d[:, co:co + 1, :])
                    nc.sync.dma_start(out=xt_sb[:, co:co + 1, :],
                                      in_=x_d[:, co:co + 1, :])
                nc.sync.dma_start(out=wk_sb[:], in_=wk_d[:])
                nc.sync.dma_start(out=wv_sb[:], in_=wv_d[:])
                nc.sync.dma_start(out=kc_sb[:], in_=kc_d[:])
                nc.sync.dma_start(out=vc_sb[:], in_=vc_d[:])
                nc.sync.dma_start(out=mk_sb[:], in_=mk_d[:])

                # Q first half: co-outer over 8 concurrently-open psum groups,
                # paced by the x-chunk DMA arrivals (own scoped pool so the 8
                # banks are returned before the main psum pool allocates)
                with tc.tile_pool(name="psum8", bufs=1, space="PSUM") as psum8:
                    groups8 = [(h, ti) for h in range(HL_) for ti in range(2)]
                    pps8 = {}
                    for g in groups8:
                        pps8[g] = psum8.tile([P, TCH], F32, tag="mm8", bufs=8,
                                             name="pp8")
                    for co in range(CO):
                        for (h, ti) in groups8:
                            nc.tensor.matmul(
                                pps8[(h, ti)][:],
                                wq_sb[:, co, h * HD:(h + 1) * HD],
                                xt_sb[:, co, ti * TCH:(ti + 1) * TCH],
                                start=(co == 0),
                                stop=(co == CO - 1),
                            )
                    for (h, ti) in groups8:
                        nc.scalar.copy(out=qt_sb[:, h, ti * TCH:(ti + 1) * TCH],
                                       in_=pps8[(h, ti)][:])

                with tc.tile_pool(name="psumA", bufs=1, space="PSUM") as psumA:
                    # Q second half and K: co-inner per group (x resident)
                    qk_rest = [(wq_sb, qt_sb, h, ti)
                               for h in range(HL_) for ti in (2, 3)]
                    qk_rest += [(wk_sb, kt_sb, h, ti)
                                for h in range(HL_) for ti in range(NT)]
                    for wsb, dst, h, ti in qk_rest:
                        pp = psumA.tile([P, TCH], F32, tag="mm", bufs=2,
                                        name="pp_qk")
                        for co in range(CO):
                            nc.tensor.matmul(
                                pp[:],
                                wsb[:, co, h * HD:(h + 1) * HD],
                                xt_sb[:, co, ti * TCH:(ti + 1) * TCH],
                                start=(co == 0),
                                stop=(co == CO - 1),
                            )
                        nc.scalar.copy(
                            out=dst[:, h, ti * TCH:(ti + 1) * TCH], in_=pp[:]
                        )

                    # V in [t, d] layout: psum [t=128, d=DL]
                    for tt in range(T_ // P):
                        pv = psumA.tile([P, DL], F32, tag="mm", bufs=2,
                                        name="pv")
                        for co in range(CO):
                            nc.tensor.matmul(
                                pv[:],
                                xt_sb[:, co, tt * P:(tt + 1) * P],
                                wv_sb[:, co, :],
                                start=(co == 0),
                                stop=(co == CO - 1),
                            )
                        nc.scalar.copy(out=vn_sb[:, tt, :], in_=pv[:])

            # ---- phase B: attention + output projection ----
            with (
                tc.tile_pool(name="work", bufs=1) as work,
                tc.tile_pool(name="psum", bufs=1, space="PSUM") as psum,
                tc.tile_pool(name="dscr", bufs=2, space="DRAM") as dscr,
            ):
                wo_sb = work.tile([P, HL_, C_], BF16, name="wo_sb")
                nc.sync.dma_start(out=wo_sb[:], in_=wo_d[:])

                def emit_y(ti):
                    # y[t, c] += onorm^T(h-chunks) @ wo ; psum DMA'd straight out
                    onorm = onorms[ti]
                    for ci in range(NCC):
                        for tsub in range(TSUB):
                            py = psum.tile([P, TCH], F32, tag="mm", bufs=2,
                                           name="py")
                            for h in range(HL_):
                                nc.tensor.matmul(
                                    py[:],
                                    onorm[:, h, tsub * P:(tsub + 1) * P],
                                    wo_sb[:, h, ci * TCH:(ci + 1) * TCH],
                                    start=(h == 0),
                                    stop=(h == HL_ - 1),
                                )
                            ysb = work.tile([P, TCH], F32, tag="ysb", bufs=3,
                                            name="ysb")
                            nc.vector.tensor_copy(out=ysb[:], in_=py[:])
                            t0 = ti * TCH + tsub * P
                            nc.sync.dma_start(
                                out=y_d[t0:t0 + P, ci * TCH:(ci + 1) * TCH],
                                in_=ysb[:],
                            )

                onorms = {}
                for ti in range(NT):
                    onorm = work.tile([P, HL_, TCH], BF16, tag="onorm", bufs=2,
                                      name="onorm")
                    onorms[ti] = onorm
                    den4 = work.tile([P, TCH], F32, tag="den4", bufs=2,
                                     name="den4")
                    nc.vector.memset(den4[:], 1.0)
                    osbs = {}
                    for h in range(HL_):
                        n_new = (ti + 1) * TSUB
                        n_s = SCO + n_new
                        n_pair = n_s // 2
                        diag0 = n_s - TSUB  # first diagonal (masked) chunk
                        q_rhs = qt_sb[:, h, ti * TCH:(ti + 1) * TCH]

                        acc = psum.tile([P, TCH], F32, tag="acc", bufs=2,
                                        name="acc")
                        # running denominator partial-sum (bf16, DVE-updated;
                        # GpSimd shares SBUF ports with DVE — keep it idle)
                        pacc = work.tile([P, TCH], BF16, tag="pacc", bufs=2,
                                         name="pacc")

                        def c0_of(j):
                            # first needed query column for key chunk j
                            # (causality: chunk at diag offset m only feeds
                            #  queries t >= 128*m)
                            return 0 if j < diag0 else P * (j - diag0)

                        def kt_of(j):
                            if j < SCO:
                                return kc_sb[:, h, j, :]
                            sn = j - SCO
                            return kt_sb[:, h, sn * P:(sn + 1) * P]

                        def v_of(j):
                            if j < SCO:
                                return vc_sb[:, j, h, :]
                            sn = j - SCO
                            return vn_sb[:, sn, h * HD:(h + 1) * HD]

                        def qk_pair(p):
                            s2 = psum.tile([P, 2, TCH], F32, tag="S2", bufs=2,
                                           name="s2")
                            for i in (0, 1):
                                j = 2 * p + i
                                c0 = c0_of(j)
                                nc.tensor.matmul(s2[:, i, c0:], kt_of(j),
                                                 q_rhs[:, c0:],
                                                 start=True, stop=True)
                            return s2

                        def consume_pair(p, s2):
                            e2 = work.tile([P, 2, TCH], BF16, tag="E", bufs=3,
                                           name="e2")
                            j0, j1 = 2 * p, 2 * p + 1
                            if c0_of(j1) == 0:
                                nc.scalar.activation(
                                    out=e2[:], in_=s2[:],
                                    func=mybir.ActivationFunctionType.Exp,
                                )
                            else:
                                for i, j in ((0, j0), (1, j1)):
                                    c0 = c0_of(j)
                                    nc.scalar.activation(
                                        out=e2[:, i, c0:], in_=s2[:, i, c0:],
                                        func=mybir.ActivationFunctionType.Exp,
                                    )
                            for i, j in ((0, j0), (1, j1)):
                                m = j - diag0
                                if m >= 0:
                                    # only the 128-wide diagonal block needs
                                    # masking; columns beyond it are all-ones
                                    c0 = P * m
                                    nc.vector.tensor_mul(
                                        e2[:, i, c0:c0 + P], e2[:, i, c0:c0 + P],
                                        mk_sb[:, m, c0:c0 + P])
                            for i, j in ((0, j0), (1, j1)):
                                c0 = c0_of(j)
                                if j == 0:
                                    nc.vector.tensor_copy(out=pacc[:],
                                                          in_=e2[:, 0, :])
                                else:
                                    nc.vector.tensor_add(pacc[:, c0:],
                                                         pacc[:, c0:],
                                                         e2[:, i, c0:])
                                nc.tensor.matmul(
                                    acc[:, c0:], v_of(j), e2[:, i, c0:],
                                    start=(j == 0), stop=(j == n_s - 1),
                                )

                        # software pipeline: QK(p+1) issued before consuming p
                        s2_prev = qk_pair(0)
                        for p in range(n_pair):
                            s2_next = qk_pair(p + 1) if p + 1 < n_pair else None
                            consume_pair(p, s2_prev)
                            s2_prev = s2_next

                        # release acc early: unnormalized output to sbuf
                        osb = work.tile([P, TCH], BF16, tag="osb", bufs=8,
                                        name="osb")
                        nc.vector.tensor_copy(out=osb[:], in_=acc[:])
                        osbs[h] = osb

                        # denominator: single ones-matmul over the running sum
                        dps = psum.tile([1, TCH], F32, tag="mm", bufs=2,
                                        name="dps")
                        nc.tensor.matmul(dps[:1, :], ones_col[:], pacc[:],
                                         start=True, stop=True)
                        nc.scalar.copy(out=den4[32 * h:32 * h + 1, :], in_=dps[:1, :])

                    # normalization chain issued BEFORE emit_y(ti-1) so its
                    # DMA broadcast roundtrip hides under the py matmuls
                    recip4 = work.tile([P, TCH], F32, tag="recip4", bufs=2,
                                       name="recip4")
                    nc.vector.reciprocal(out=recip4[:], in_=den4[:])
                    # gather the 4 rows (partitions 0/32/64/96) to DRAM, then
                    # one broadcast read back (0-stride partition APs need DRAM)
                    rdr4 = dscr.tile([HL_, TCH], F32, tag="rdr4", bufs=2,
                                     name="rdr4")
                    nc.sync.dma_start(out=rdr4[:],
                                      in_=recip4[:32 * HL_:32, :])
                    rbc4 = work.tile([P, HL_, TCH], F32, tag="rbc4", bufs=2,
                                     name="rbc4")
                    bcast_src = bass.AP(
                        tensor=rdr4.tensor, offset=rdr4.offset,
                        ap=[[0, P], [TCH, HL_], [1, TCH]],
                    )
                    nc.sync.dma_start(out=rbc4[:], in_=bcast_src)
                    for h in range(HL_):
                        nc.vector.tensor_mul(onorm[:, h, :], osbs[h][:],
                                             rbc4[:, h, :])

                    # y of the previous t-chunk: fills PE while this chunk's
                    # normalization tail drains on DVE/DMA
                    if ti > 0:
                        emit_y(ti - 1)
                emit_y(NT - 1)

    # walrus allows a single sync wait per hw instruction: shed matmul extras
    # onto ldweights, then split any remaining multi-waits via event sems
    bass._bass_rust.move_matmul_waits_to_ldweights(nc.m)
    bass._bass_rust.generate_event_semaphores(nc)
    return nc


def _bf16(a):
    return np.ascontiguousarray(a).astype(ml_dtypes.bfloat16)


def make_core_inputs(x, k_cache, v_cache, wq, wk, wv, wo, core,
                     T_=T, C_=C, HL_=HL, SC_=START, n_groups=None):
    """Host-side shard + relayout for one core."""
    CO = C_ // P
    DL = HL_ * HD
    TSUB = TCH // P
    SCO = SC_ // P
    if n_groups is None:
        n_groups = (k_cache.shape[1] + HL_ - 1) // HL_
    b, g = divmod(core, n_groups)
    heads = slice(HL_ * g, HL_ * (g + 1))
    rows = slice(DL * g, DL * (g + 1))
    scale = HD ** -0.5

    xd = x[b].T.reshape(CO, P, T_).transpose(1, 0, 2)
    wqd = (wq[rows].T * scale).reshape(CO, P, DL).transpose(1, 0, 2)
    wkd = wk[rows].T.reshape(CO, P, DL).transpose(1, 0, 2)
    wvd = wv[rows].T.reshape(CO, P, DL).transpose(1, 0, 2)
    wod = wo[:, rows].T.reshape(HL_, P, C_).transpose(1, 0, 2)
    kcd = k_cache[b, heads].reshape(HL_, SCO, P, P).transpose(3, 0, 1, 2)
    vcd = v_cache[b, heads].reshape(HL_, SCO, P, P).transpose(2, 1, 0, 3)
    si = np.arange(P)[:, None, None]
    mm = np.arange(TSUB)[None, :, None]
    tt = np.arange(TCH)[None, None, :]
    mkd = (tt >= si + P * mm)

    return {
        "x": _bf16(xd), "wq": _bf16(wqd), "wk": _bf16(wkd), "wv": _bf16(wvd),
        "wo": _bf16(wod), "kc": _bf16(kcd), "vc": _bf16(vcd),
        "mk": _bf16(mkd.astype(np.float32)),
    }


_NC_CACHE = None


def _get_nc():
    global _NC_CACHE
    if _NC_CACHE is None:
        _NC_CACHE = build_nc()
    return _NC_CACHE


def run_spmd(inputs, trace=False):
    """Run the 8-core SPMD kernel; returns (y_full, BassKernelResults)."""
    from concourse.bass_utils import run_bass_kernel_spmd

    x = np.asarray(inputs["x"], dtype=np.float32)
    k_cache = np.asarray(inputs["k_cache"], dtype=np.float32)
    v_cache = np.asarray(inputs["v_cache"], dtype=np.float32)
    wq = np.asarray(inputs["wq"], dtype=np.float32)
    wk = np.asarray(inputs["wk"], dtype=np.float32)
    wv = np.asarray(inputs["wv"], dtype=np.float32)
    wo = np.asarray(inputs["wo"], dtype=np.float32)
    assert int(inputs["start_pos"]) == START

    nc = _get_nc()
    in_maps = [
        make_core_inputs(x, k_cache, v_cache, wq, wk, wv, wo, core)
        for core in range(N_CORES)
    ]
    res = run_bass_kernel_spmd(
        nc, in_maps, core_ids=list(range(N_CORES)), trace=trace
    )
    n_groups = N_CORES // B
    y = np.zeros((B, T, C), dtype=np.float32)
    for core in range(N_CORES):
        b = core // n_groups
        y[b] += np.asarray(res.results[core]["y"], dtype=np.float32)
    return y, res


def kernel(**inputs):
    y, _ = run_spmd(inputs, trace=False)
    return y



# revision 26
# speedup vs baseline: 1.1150x; 1.0605x over previous
"""Causal attention with KV cache — Trainium2 Bass kernel, 8-core SPMD.

Sharding: batch (2) x head-group (4 heads each) = 8 cores.
Each core computes, for its (batch b, heads 4g..4g+3):
  Q/K/V projections (bf16 matmuls, fp32 psum),
  streaming softmax(QK^T)V with the KV cache (no max subtraction -- scores
  are O(5) here so exp is safe), and its partial output projection
  y_partial = attn_out @ wo[:, heads].T  (shape [T, C], fp32).
Host sums the 4 head-group partials per batch.

Device data layouts (host pre-arranges everything, bf16):
  x    [128 ci, CO co, T t]        = x[b].T  split c=(co ci)
  wq/wk/wv [128 ci, CO co, DL d]   = w[rows].T split c, (wq pre-scaled by hd^-0.5)
  wo   [128 di, HL do, C c]        = wo[:, rows].T split d=(do di)
  kc   [128 d, HL h, SCO so, 128 si]
  vc   [128 si, SCO so, HL h, 128 d]
  mk   [128 si, 4 m, 512 t]        causal masks for the 4 diagonal chunks
Output:
  y    [T, C] fp32 (partial sum over this core's heads)
"""

import os
import sys

import numpy as np

for _p in ("/opt/trn_rl_repo", os.path.expanduser("~/.axon_site/_ro/trn_rl_repo")):
    if _p not in sys.path and os.path.isdir(_p):
        sys.path.insert(0, _p)

import ml_dtypes  # noqa: E402

import concourse.bass as bass  # noqa: E402
import concourse.tile as tile  # noqa: E402
from concourse import mybir  # noqa: E402

BF16 = mybir.dt.bfloat16
F32 = mybir.dt.float32
P = 128

# Full-size problem constants
B, T, C, H, HD, START = 2, 2048, 2048, 16, 128, 1024
N_CORES = 8
N_GROUPS = N_CORES // B     # 4 head groups per batch
HL = H // N_GROUPS          # 4 local heads per core
TCH = 512  # t-chunk (psum free dim)


def build_nc(T_=T, C_=C, HL_=HL, SC_=START):
    """Build the per-core Bass module. All cores run the identical program."""
    nc = bass.Bass("TRN2", target_bir_lowering=False)

    CO = C_ // P            # contraction chunks for projections
    DL = HL_ * HD           # local head dims (512)
    NT = T_ // TCH          # query t-chunks
    TSUB = TCH // P         # 128-row subchunks per t-chunk (= #diagonal masks)
    SCO = SC_ // P          # cache s-chunks
    NCC = C_ // TCH         # output column chunks

    x_d = nc.dram_tensor("x", [P, CO, T_], BF16, kind="ExternalInput")
    wq_d = nc.dram_tensor("wq", [P, CO, DL], BF16, kind="ExternalInput")
    wk_d = nc.dram_tensor("wk", [P, CO, DL], BF16, kind="ExternalInput")
    wv_d = nc.dram_tensor("wv", [P, CO, DL], BF16, kind="ExternalInput")
    wo_d = nc.dram_tensor("wo", [P, HL_, C_], BF16, kind="ExternalInput")
    kc_d = nc.dram_tensor("kc", [P, HL_, SCO, P], BF16, kind="ExternalInput")
    vc_d = nc.dram_tensor("vc", [P, SCO, HL_, P], BF16, kind="ExternalInput")
    mk_d = nc.dram_tensor("mk", [P, TSUB, TCH], BF16, kind="ExternalInput")
    y_d = nc.dram_tensor("y", [T_, C_], F32, kind="ExternalOutput")

    F32R = mybir.dt.float32r
    with tile.TileContext(nc) as tc:
        with (
            tc.tile_pool(name="consts", bufs=1) as consts,
        ):
            # ---- persistent SBUF state (alive across both phases) ----
            ones_col = consts.tile([P, 1], BF16)
            nc.vector.memset(ones_col[:], 1.0)

            qt_sb = consts.tile([P, HL_, T_], BF16)   # Q^T  [d, h, t]
            kt_sb = consts.tile([P, HL_, T_], BF16)   # K^T new  [d, h, t]
            vn_sb = consts.tile([P, T_ // P, DL], BF16)  # V new  [t_i, t_o, d]
            kc_sb = consts.tile([P, HL_, SCO, P], BF16)
            vc_sb = consts.tile([P, SCO, HL_, P], BF16)
            mk_sb = consts.tile([P, TSUB, TCH], BF16)

            # ---- phase A: projections (x and wq/wk/wv freed afterwards) ----
            with tc.tile_pool(name="projp", bufs=1) as projp:
                xt_sb = projp.tile([P, CO, T_], BF16)
                wq_sb = projp.tile([P, CO, DL], BF16)
                wk_sb = projp.tile([P, CO, DL], BF16)
                wv_sb = projp.tile([P, CO, DL], BF16)
                # interleave wq/x chunk DMAs so the co-outer first Q sweep can
                # begin after chunk 0 lands rather than after the full 10MB
                for co in range(CO):
                    nc.sync.dma_start(out=wq_sb[:, co:co + 1, :],
                                      in_=wq_d[:, co:co + 1, :])
                    nc.sync.dma_start(out=xt_sb[:, co:co + 1, :],
                                      in_=x_d[:, co:co + 1, :])
                nc.sync.dma_start(out=wk_sb[:], in_=wk_d[:])
                nc.sync.dma_start(out=wv_sb[:], in_=wv_d[:])
                nc.sync.dma_start(out=kc_sb[:], in_=kc_d[:])
                nc.sync.dma_start(out=vc_sb[:], in_=vc_d[:])
                nc.sync.dma_start(out=mk_sb[:], in_=mk_d[:])

                # Q first half: co-outer over 8 concurrently-open psum groups,
                # paced by the x-chunk DMA arrivals (own scoped pool so the 8
                # banks are returned before the main psum pool allocates)
                with tc.tile_pool(name="psum8", bufs=1, space="PSUM") as psum8:
                    groups8 = [(h, ti) for h in range(HL_) for ti in range(2)]
                    pps8 = {}
                    for g in groups8:
                        pps8[g] = psum8.tile([P, TCH], F32, tag="mm8", bufs=8,
                                             name="pp8")
                    for co in range(CO):
                        for (h, ti) in groups8:
                            nc.tensor.matmul(
                                pps8[(h, ti)][:],
                                wq_sb[:, co, h * HD:(h + 1) * HD],
                                xt_sb[:, co, ti * TCH:(ti + 1) * TCH],
                                start=(co == 0),
                                stop=(co == CO - 1),
                            )
                    for (h, ti) in groups8:
                        nc.scalar.copy(out=qt_sb[:, h, ti * TCH:(ti + 1) * TCH],
                                       in_=pps8[(h, ti)][:])

                with tc.tile_pool(name="psumA", bufs=1, space="PSUM") as psumA:
                    # Q second half and K: co-inner per group (x resident)
                    qk_rest = [(wq_sb, qt_sb, h, ti)
                               for h in range(HL_) for ti in (2, 3)]
                    qk_rest += [(wk_sb, kt_sb, h, ti)
                                for h in range(HL_) for ti in range(NT)]
                    for wsb, dst, h, ti in qk_rest:
                        pp = psumA.tile([P, TCH], F32, tag="mm", bufs=2,
                                        name="pp_qk")
                        for co in range(CO):
                            nc.tensor.matmul(
                                pp[:],
                                wsb[:, co, h * HD:(h + 1) * HD],
                                xt_sb[:, co, ti * TCH:(ti + 1) * TCH],
                                start=(co == 0),
                                stop=(co == CO - 1),
                            )
                        nc.scalar.copy(
                            out=dst[:, h, ti * TCH:(ti + 1) * TCH], in_=pp[:]
                        )

                    # V in [t, d] layout: psum [t=128, d=DL]
                    for tt in range(T_ // P):
                        pv = psumA.tile([P, DL], F32, tag="mm", bufs=2,
                                        name="pv")
                        for co in range(CO):
                            nc.tensor.matmul(
                                pv[:],
                                xt_sb[:, co, tt * P:(tt + 1) * P],
                                wv_sb[:, co, :],
                                start=(co == 0),
                                stop=(co == CO - 1),
                            )
                        nc.scalar.copy(out=vn_sb[:, tt, :], in_=pv[:])

            # ---- phase B: attention + output projection ----
            with (
                tc.tile_pool(name="work", bufs=1) as work,
                tc.tile_pool(name="psum", bufs=1, space="PSUM") as psum,
                tc.tile_pool(name="dscr", bufs=2, space="DRAM") as dscr,
            ):
                wo_sb = work.tile([P, HL_, C_], BF16, name="wo_sb")
                nc.sync.dma_start(out=wo_sb[:], in_=wo_d[:])

                def emit_y_group(ti, ci, tsub):
                    # one [128t, 512c] tile of y += onorm^T @ wo
                    onorm = onorms[ti]
                    py = psum.tile([P, TCH], F32, tag="mm", bufs=2,
                                   name="py")
                    for h in range(HL_):
                        nc.tensor.matmul(
                            py[:],
                            onorm[:, h, tsub * P:(tsub + 1) * P],
                            wo_sb[:, h, ci * TCH:(ci + 1) * TCH],
                            start=(h == 0),
                            stop=(h == HL_ - 1),
                        )
                    ysb = work.tile([P, TCH], F32, tag="ysb", bufs=3,
                                    name="ysb")
                    nc.vector.tensor_copy(out=ysb[:], in_=py[:])
                    t0 = ti * TCH + tsub * P
                    nc.sync.dma_start(
                        out=y_d[t0:t0 + P, ci * TCH:(ci + 1) * TCH],
                        in_=ysb[:],
                    )

                # emit queue: previous t-chunk's y tiles, drizzled into the
                # attention pair loop as independent PE filler work
                emit_q = []

                def drain_emit(k):
                    for _ in range(min(k, len(emit_q))):
                        emit_y_group(*emit_q.pop(0))

                onorms = {}
                for ti in range(NT):
                    if ti > 0:
                        emit_q.extend((ti - 1, ci, tsub)
                                      for ci in range(NCC)
                                      for tsub in range(TSUB))
                    onorm = work.tile([P, HL_, TCH], BF16, tag="onorm", bufs=2,
                                      name="onorm")
                    onorms[ti] = onorm
                    den4 = work.tile([P, TCH], F32, tag="den4", bufs=2,
                                     name="den4")
                    nc.vector.memset(den4[:], 1.0)
                    osbs = {}
                    n_new = (ti + 1) * TSUB
                    n_s = SCO + n_new
                    n_pair = n_s // 2
                    diag0 = n_s - TSUB  # first diagonal (masked) chunk

                    def c0_of(j):
                        # first needed query column for key chunk j
                        # (causality: chunk at diag offset m only feeds
                        #  queries t >= 128*m)
                        return 0 if j < diag0 else P * (j - diag0)

                    def kt_of(j, h):
                        if j < SCO:
                            return kc_sb[:, h, j, :]
                        sn = j - SCO
                        return kt_sb[:, h, sn * P:(sn + 1) * P]

                    def v_of(j, h):
                        if j < SCO:
                            return vc_sb[:, j, h, :]
                        sn = j - SCO
                        return vn_sb[:, sn, h * HD:(h + 1) * HD]

                    def qk_pair(p, h):
                        s2 = psum.tile([P, 2, TCH], F32, tag="S2", bufs=2,
                                       name="s2")
                        q_rhs = qt_sb[:, h, ti * TCH:(ti + 1) * TCH]
                        for i in (0, 1):
                            j = 2 * p + i
                            c0 = c0_of(j)
                            nc.tensor.matmul(s2[:, i, c0:], kt_of(j, h),
                                             q_rhs[:, c0:],
                                             start=True, stop=True)
                        return s2

                    def consume_pair(p, s2, h, acc, pacc2):
                        e2 = work.tile([P, 2, TCH], BF16, tag="E", bufs=3,
                                       name="e2")
                        j0, j1 = 2 * p, 2 * p + 1
                        if c0_of(j1) == 0:
                            nc.scalar.activation(
                                out=e2[:], in_=s2[:],
                                func=mybir.ActivationFunctionType.Exp,
                            )
                        else:
                            for i, j in ((0, j0), (1, j1)):
                                c0 = c0_of(j)
                                nc.scalar.activation(
                                    out=e2[:, i, c0:], in_=s2[:, i, c0:],
                                    func=mybir.ActivationFunctionType.Exp,
                                )
                        for i, j in ((0, j0), (1, j1)):
                            m = j - diag0
                            if m >= 0:
                                # only the 128-wide diagonal block needs
                                # masking; columns beyond it are all-ones
                                c0 = P * m
                                nc.vector.tensor_mul(
                                    e2[:, i, c0:c0 + P], e2[:, i, c0:c0 + P],
                                    mk_sb[:, m, c0:c0 + P])
                        # denominator partial-sums: one double-width DVE add
                        # per pair (halves into pacc2[:,0,:] / pacc2[:,1,:])
                        if j0 == 0:
                            nc.vector.tensor_copy(out=pacc2[:], in_=e2[:])
                        elif c0_of(j1) == 0:
                            nc.vector.tensor_add(pacc2[:], pacc2[:], e2[:])
                        else:
                            for i, j in ((0, j0), (1, j1)):
                                c0 = c0_of(j)
                                nc.vector.tensor_add(pacc2[:, i, c0:],
                                                     pacc2[:, i, c0:],
                                                     e2[:, i, c0:])
                        for i, j in ((0, j0), (1, j1)):
                            c0 = c0_of(j)
                            nc.tensor.matmul(
                                acc[:, c0:], v_of(j, h), e2[:, i, c0:],
                                start=(j == 0), stop=(j == n_s - 1),
                            )

                    s2_carry = None
                    for h in range(HL_):
                        acc = psum.tile([P, TCH], F32, tag="acc", bufs=2,
                                        name="acc")
                        # running denominator partial-sums (bf16, DVE-updated;
                        # GpSimd shares SBUF ports with DVE — keep it idle)
                        pacc2 = work.tile([P, 2, TCH], BF16, tag="pacc", bufs=2,
                                          name="pacc")

                        # software pipeline: QK(p+1) issued before consuming p,
                        # crossing into the next head at the boundary
                        s2_prev = s2_carry if s2_carry is not None \
                            else qk_pair(0, h)
                        s2_carry = None
                        for p in range(n_pair):
                            if p + 1 < n_pair:
                                s2_next = qk_pair(p + 1, h)
                            elif h + 1 < HL_:
                                s2_next = qk_pair(0, h + 1)
                                s2_carry = s2_next
                            else:
                                s2_next = None
                            consume_pair(p, s2_prev, h, acc, pacc2)
                            # drizzle previous t-chunk's y matmuls into the
                            # stream as exp-latency filler for the PE
                            if p % 2 == 1:
                                drain_emit(1)
                            s2_prev = s2_next

                        # release acc early: unnormalized output to sbuf
                        osb = work.tile([P, TCH], BF16, tag="osb", bufs=8,
                                        name="osb")
                        nc.vector.tensor_copy(out=osb[:], in_=acc[:])
                        osbs[h] = osb

                        # denominator: ones-matmuls over the two running sums
                        dps = psum.tile([1, TCH], F32, tag="mm", bufs=2,
                                        name="dps")
                        nc.tensor.matmul(dps[:1, :], ones_col[:],
                                         pacc2[:, 0, :], start=True, stop=False)
                        nc.tensor.matmul(dps[:1, :], ones_col[:],
                                         pacc2[:, 1, :], start=False, stop=True)
                        nc.scalar.copy(out=den4[32 * h:32 * h + 1, :], in_=dps[:1, :])

                    # normalization chain issued BEFORE emit_y(ti-1) so its
                    # DMA broadcast roundtrip hides under the py matmuls
                    recip4 = work.tile([P, TCH], F32, tag="recip4", bufs=2,
                                       name="recip4")
                    nc.vector.reciprocal(out=recip4[:], in_=den4[:])
                    # gather the 4 rows (partitions 0/32/64/96) to DRAM, then
                    # one broadcast read back (0-stride partition APs need DRAM)
                    rdr4 = dscr.tile([HL_, TCH], F32, tag="rdr4", bufs=2,
                                     name="rdr4")
                    nc.sync.dma_start(out=rdr4[:],
                                      in_=recip4[:32 * HL_:32, :])
                    rbc4 = work.tile([P, HL_, TCH], F32, tag="rbc4", bufs=2,
                                     name="rbc4")
                    bcast_src = bass.AP(
                        tensor=rdr4.tensor, offset=rdr4.offset,
                        ap=[[0, P], [TCH, HL_], [1, TCH]],
                    )
                    nc.sync.dma_start(out=rbc4[:], in_=bcast_src)
                    for h in range(HL_):
                        nc.vector.tensor_mul(onorm[:, h, :], osbs[h][:],
                                             rbc4[:, h, :])

                    # flush any y tiles not drained inside the pair loops
                    drain_emit(len(emit_q))
                # final t-chunk's y
                emit_q.extend((NT - 1, ci, tsub)
                              for ci in range(NCC) for tsub in range(TSUB))
                drain_emit(len(emit_q))

    # walrus allows a single sync wait per hw instruction: shed matmul extras
    # onto ldweights, then split any remaining multi-waits via event sems
    bass._bass_rust.move_matmul_waits_to_ldweights(nc.m)
    bass._bass_rust.generate_event_semaphores(nc)
    return nc


def _bf16(a):
    return np.ascontiguousarray(a).astype(ml_dtypes.bfloat16)


def make_core_inputs(x, k_cache, v_cache, wq, wk, wv, wo, core,
                     T_=T, C_=C, HL_=HL, SC_=START, n_groups=None):
    """Host-side shard + relayout for one core."""
    CO = C_ // P
    DL = HL_ * HD
    TSUB = TCH // P
    SCO = SC_ // P
    if n_groups is None:
        n_groups = (k_cache.shape[1] + HL_ - 1) // HL_
    b, g = divmod(core, n_groups)
    heads = slice(HL_ * g, HL_ * (g + 1))
    rows = slice(DL * g, DL * (g + 1))
    scale = HD ** -0.5

    xd = x[b].T.reshape(CO, P, T_).transpose(1, 0, 2)
    wqd = (wq[rows].T * scale).reshape(CO, P, DL).transpose(1, 0, 2)
    wkd = wk[rows].T.reshape(CO, P, DL).transpose(1, 0, 2)
    wvd = wv[rows].T.reshape(CO, P, DL).transpose(1, 0, 2)
    wod = wo[:, rows].T.reshape(HL_, P, C_).transpose(1, 0, 2)
    kcd = k_cache[b, heads].reshape(HL_, SCO, P, P).transpose(3, 0, 1, 2)
    vcd = v_cache[b, heads].reshape(HL_, SCO, P, P).transpose(2, 1, 0, 3)
    si = np.arange(P)[:, None, None]
    mm = np.arange(TSUB)[None, :, None]
    tt = np.arange(TCH)[None, None, :]
    mkd = (tt >= si + P * mm)

    return {
        "x": _bf16(xd), "wq": _bf16(wqd), "wk": _bf16(wkd), "wv": _bf16(wvd),
        "wo": _bf16(wod), "kc": _bf16(kcd), "vc": _bf16(vcd),
        "mk": _bf16(mkd.astype(np.float32)),
    }


_NC_CACHE = None


def _get_nc():
    global _NC_CACHE
    if _NC_CACHE is None:
        _NC_CACHE = build_nc()
    return _NC_CACHE


def run_spmd(inputs, trace=False):
    """Run the 8-core SPMD kernel; returns (y_full, BassKernelResults)."""
    from concourse.bass_utils import run_bass_kernel_spmd

    x = np.asarray(inputs["x"], dtype=np.float32)
    k_cache = np.asarray(inputs["k_cache"], dtype=np.float32)
    v_cache = np.asarray(inputs["v_cache"], dtype=np.float32)
    wq = np.asarray(inputs["wq"], dtype=np.float32)
    wk = np.asarray(inputs["wk"], dtype=np.float32)
    wv = np.asarray(inputs["wv"], dtype=np.float32)
    wo = np.asarray(inputs["wo"], dtype=np.float32)
    assert int(inputs["start_pos"]) == START

    nc = _get_nc()
    in_maps = [
        make_core_inputs(x, k_cache, v_cache, wq, wk, wv, wo, core)
        for core in range(N_CORES)
    ]
    res = run_bass_kernel_spmd(
        nc, in_maps, core_ids=list(range(N_CORES)), trace=trace
    )
    n_groups = N_CORES // B
    y = np.zeros((B, T, C), dtype=np.float32)
    for core in range(N_CORES):
        b = core // n_groups
        y[b] += np.asarray(res.results[core]["y"], dtype=np.float32)
    return y, res


def kernel(**inputs):
    y, _ = run_spmd(inputs, trace=False)
    return y

